# revision 59
# baseline (speedup 1.0000x reference)
"""Trainium2 Bass kernel for the shifted-window attention block
(nn_Block_6373731467375), SPMD over 8 NeuronCores, data-parallel over batch.

Per core: 2 batch elements. Pass A computes the attention branch in rolled
window space (LN1 folded into qkv weights, dual-S softmax: S token-major for
denominators, S feature-major for the AV matmul), writes the scaled branch
output to a DRAM scratch in original token order. Pass B adds the residual,
applies LN2 (folded into fc1), runs the MLP and writes the final output.
"""

import numpy as np
import ml_dtypes

BF = ml_dtypes.bfloat16

DIM, H, HD, WS, SHIFT, NPATCH, MLP, EPS = 768, 12, 64, 128, 64, 128, 3072, 1e-5
B, N = 16, 2000
NCORES = 8
BL = B // NCORES          # batch elems per core
TOK = BL * N              # 4000
NW = 16                   # rolled 128-token tiles (=windows) per batch elem
NG = 4                    # groups of 4 tiles (512 tokens)
CC = DIM // 128           # 6 contraction chunks
JB = MLP // 128           # 24 hidden blocks
MAGIC = 0x5F3759DF
# Schraudolph exp-in-bf16-bits: bits = A16*x + (127*128 - C16); sqrt(A16) is
# folded into the q/k projections so PSUM holds A16*S directly.
A16 = 128.0 / np.log(2.0)
RA16 = float(np.sqrt(A16))
K16 = 127.0 * 128.0 - 5.5

_CACHE = {}


# ---------------------------------------------------------------------------
# device kernel builder
# ---------------------------------------------------------------------------

def _fix_multi_waits(nc, mybir):
    """This walrus build rejects >1 sync-wait per instruction; hoist extra
    waits onto dedicated NOPs inserted just before, on the same engine."""
    n = 0
    for blk in nc.main_func.blocks:
        new_insts = []
        changed = False
        for ins in blk.instructions:
            si = ins.sync_info
            if si is not None and si.on_wait and len(si.on_wait) > 1:
                waits = list(si.on_wait)
                for w in waits[:-1]:
                    n += 1
                    nop = mybir.InstNoOp(
                        name=f"{ins.name}-sw{n}",
                        engine=ins.engine,
                        ins=[],
                        outs=[],
                        bass_nofuse=True,
                        sync_info=mybir.SyncInfo(on_wait=[w], on_update=[]),
                    )
                    new_insts.append(nop)
                si.on_wait = waits[-1:]
                changed = True
            new_insts.append(ins)
        if changed:
            blk.instructions = new_insts
    return n


def _build(fix_waits=True, passes=('A','B')):
    import concourse.bass as bass
    import concourse.mybir as mybir
    from contextlib import ExitStack

    f32 = mybir.dt.float32
    bf16 = mybir.dt.bfloat16
    f8 = mybir.dt.float8e4
    u32 = mybir.dt.uint32
    i16 = mybir.dt.int16
    DR = mybir.MatmulPerfMode.DoubleRow
    AX = mybir.AxisListType
    OP = mybir.AluOpType
    AF = mybir.ActivationFunctionType

    from concourse.tile import TileContext

    nc = bass.Bass()
    p = {}
    p["xs"] = nc.declare_dram_parameter("xs", [TOK, DIM], f32, isOutput=False)
    p["xbf"] = nc.declare_dram_parameter("xbf", [TOK, DIM], bf16, isOutput=False)
    p["wqk"] = nc.declare_dram_parameter("wqk", [128, 12, CC, 128], f8, isOutput=False)
    p["wv"] = nc.declare_dram_parameter("wv", [128, CC, DIM], f8, isOutput=False)
    p["wproj"] = nc.declare_dram_parameter("wproj", [128, CC, DIM], f8, isOutput=False)
    p["wfc1"] = nc.declare_dram_parameter("wfc1", [128, CC, MLP], f8, isOutput=False)
    p["wfc2"] = nc.declare_dram_parameter("wfc2", [128, JB, DIM], f8, isOutput=False)
    p["bqk"] = nc.declare_dram_parameter("bqk", [128, 12], f32, isOutput=False)
    p["bfc1"] = nc.declare_dram_parameter("bfc1", [128, JB], f32, isOutput=False)
    p["bproj"] = nc.declare_dram_parameter("bproj", [128, DIM], f32, isOutput=False)
    p["bfc2x"] = nc.declare_dram_parameter("bfc2x", [128, DIM], f32, isOutput=False)
    p["bfeat"] = nc.declare_dram_parameter("bfeat", [128, 2, 6, WS], bf16, isOutput=False)
    p["bfeatm"] = nc.declare_dram_parameter("bfeatm", [128, 2, 6, WS], bf16, isOutput=False)
    p["ident"] = nc.declare_dram_parameter("ident", [128, 128], bf16, isOutput=False)
    p["ident8"] = nc.declare_dram_parameter("ident8", [128, 128], f8, isOutput=False)
    out_t = nc.declare_dram_parameter("out", [TOK, DIM], f32, isOutput=True)

    with TileContext(nc) as tc, ExitStack() as ctx:
        cpool = ctx.enter_context(tc.tile_pool(name="consts", bufs=1))

        # resident constants; only the tiny first-needed ones are fetched up
        # front — the window/MLP tables are DMA'd from inside emit_A(0,0)
        # after the first group's x tiles, so the startup burst stays small.
        sb = {}
        for name in ("bqk",):
            t = cpool.tile(list(p[name].shape), p[name].dtype, tag=name)
            nc.sync.dma_start(out=t[:], in_=p[name][:])
            sb[name] = t
        for name in ("bfeat", "bfeatm", "bproj", "bfc1", "bfc2x"):
            sb[name] = cpool.tile(
                list(p[name].shape), p[name].dtype, tag=name, name=name
            )
        magic = cpool.tile([128, 1], u32, tag="magic")
        nc.vector.memset(magic[:], MAGIC)

        wB = ctx.enter_context(tc.tile_pool(name="wB", bufs=1))
        wfc1 = wB.tile([128, CC, MLP], f8)
        wfc2 = wB.tile([128, JB, DIM], f8)
        # attention branch output, resident in rolled token order (f8: the
        # branch is ls1-scaled by 1e-5, so 6% quantization is invisible)
        asb = wB.tile([128, 2 * NW, DIM], f8)

        # ---------------- helpers ----------------
        def newton_rsqrt(pool, var_view, rstdg, tagp):
            """rstdg[:, :NG] = rsqrt(var_view + eps) via 3 fp32 Newton steps."""
            vts = pool.tile([128, NG], f32, tag=tagp + "v")
            y = pool.tile([128, NG], f32, tag=tagp + "y")
            t1 = pool.tile([128, NG], f32, tag=tagp + "t")
            nc.vector.tensor_scalar_add(out=vts[:], in0=var_view, scalar1=EPS)
            nc.vector.tensor_scalar(
                out=y[:].bitcast(u32),
                in0=vts[:].bitcast(u32),
                scalar1=1,
                scalar2=None,
                op0=OP.logical_shift_right,
            )
            nc.vector.tensor_tensor(
                out=y[:].bitcast(u32),
                in0=magic[:].to_broadcast([128, NG]),
                in1=y[:].bitcast(u32),
                op=OP.subtract,
            )
            a, b = y, rstdg
            for _ in range(1):
                nc.vector.tensor_mul(out=t1[:], in0=a[:], in1=a[:])
                nc.vector.tensor_mul(out=t1[:], in0=t1[:], in1=vts[:])
                nc.vector.tensor_scalar(
                    out=t1[:], in0=t1[:], scalar1=-0.5, scalar2=1.5,
                    op0=OP.mult, op1=OP.add,
                )
                nc.vector.tensor_mul(out=b[:], in0=a[:], in1=t1[:])
                a, b = b, a
            assert a is rstdg  # odd iteration count lands in caller's tile

        def transpose6(pool, z_t, dst, tl, tpool=None):
            """z_t [128,768] bf16 -> dst[:, :, tl*128:(tl+1)*128] ([128,6,128])
            via the DMA X-bar transpose (keeps the PE out of it)."""
            zT = (tpool or pool).tile([128, CC, 128], bf16, tag="zTd")
            nc.scalar.dma_start_transpose(out=zT[:], in_=z_t[:])
            nc.scalar.activation(
                out=dst[:, :, tl * 128 : (tl + 1) * 128],
                in_=zT[:],
                func=AF.Copy,
            )

        # =================== PASSES (A/B interleaved via deps) ==========
        if True:
         with tc.tile_pool(name="wA", bufs=1) as wA, \
             tc.tile_pool(name="pa", bufs=2) as pa, \
             tc.tile_pool(name="pa1", bufs=1) as pa1, \
             tc.tile_pool(name="pa3", bufs=2) as pa3, \
             tc.tile_pool(name="pb", bufs=2) as pb, \
             tc.tile_pool(name="pb3", bufs=2) as pb3, \
             tc.tile_pool(name="pg", bufs=1) as pg, \
             tc.tile_pool(name="pxA", bufs=2, space="PSUM") as pxA, \
             tc.tile_pool(name="pxB", bufs=1, space="PSUM") as pxB, \
             tc.tile_pool(name="ps", bufs=2, space="PSUM") as ps:

            wqk = wA.tile([128, 12, CC, 128], f8)
            wv = wA.tile([128, CC, DIM], f8)
            wproj = wA.tile([128, CC, DIM], f8)

            def emit_A(b, g):
                x0 = b * N
                if True:
                    if b == 0 and g == 1:
                        # prefetch MLP weights; the dummy copies into wfc1/2
                        # give the prefetch DMAs a write-after-write dep on
                        # wv's arrival so they can't steal DMA bandwidth from
                        # the startup-critical x/qkv tiles
                        nc.gpsimd.tensor_copy(
                            out=wfc1[0:1, 0:1, 0:1], in_=wv[0:1, 0:1, 0:1]
                        )
                        nc.gpsimd.tensor_copy(
                            out=wfc2[0:1, 0:1, 0:1], in_=wv[0:1, 0:1, 0:1]
                        )
                        nc.gpsimd.dma_start(out=wfc1[:], in_=p["wfc1"][:])
                        nc.gpsimd.dma_start(out=wfc2[:], in_=p["wfc2"][:])
                    hT = pa.tile([128, CC, 512], f8, tag="hT")
                    first = b == 0 and g == 0
                    mvg = pa.tile([128, NG, 2], f32, tag="mvg")
                    rstdg = pa.tile([128, NG], f32, tag="rstdg")
                    xG = pa.tile([128, NG, DIM], bf16, tag="xG")
                    for tl in range(4):
                        t = 4 * g + tl
                        if t < NW - 1:
                            nc.sync.dma_start(
                                out=xG[:, tl, :],
                                in_=p["xbf"][x0 + 64 + 128 * t : x0 + 64 + 128 * (t + 1)],
                            )
                        else:
                            nc.vector.memset(xG[:, tl, :], 0.0)
                            nc.sync.dma_start(
                                out=xG[0:16, tl, :], in_=p["xbf"][x0 + 1984 : x0 + 2000]
                            )
                            nc.sync.dma_start(
                                out=xG[64:128, tl, :], in_=p["xbf"][x0 : x0 + 64]
                            )
                        stats = pa3.tile([128, 2, 6], f32, tag="ln_stats")
                        nc.vector.bn_stats(out=stats[:, 0, :], in_=xG[:, tl, 0:512])
                        nc.vector.bn_stats(out=stats[:, 1, :], in_=xG[:, tl, 512:768])
                        nc.vector.bn_aggr(out=mvg[:, tl, :], in_=stats[:])
                    if first:
                        # weights follow the first group's x tiles on the queues,
                        # in order of first use: qkv slices, V, window tables,
                        # proj, then the remaining bias tables.
                        for fblk in range(4):
                            nc.sync.dma_start(
                                out=wqk[:, 3 * fblk : 3 * fblk + 3, :, :],
                                in_=p["wqk"][:, 3 * fblk : 3 * fblk + 3, :, :],
                            )
                        nc.sync.dma_start(out=wv[:], in_=p["wv"][:])
                        for name in ("bfeat", "bfeatm"):
                            nc.sync.dma_start(out=sb[name][:], in_=p[name][:])
                        nc.sync.dma_start(out=wproj[:], in_=p["wproj"][:])
                        for name in ("bproj", "bfc1", "bfc2x"):
                            nc.sync.dma_start(out=sb[name][:], in_=p[name][:])
                    newton_rsqrt(pa3, mvg[:, :, 1], rstdg, "nra")
                    for tl in range(4):
                        z_t = pa3.tile([128, DIM], bf16, tag="z_t")
                        nc.vector.tensor_scalar(
                            out=z_t[:], in0=xG[:, tl, :],
                            scalar1=mvg[:, tl, 0:1], scalar2=rstdg[:, tl : tl + 1],
                            op0=OP.subtract, op1=OP.mult,
                        )
                        transpose6(pa3, z_t, hT, tl)

                    # qkv for the group (f8: feeds only the S matmuls, whose
                    # output goes through the ~3%-accurate exp trick anyway)
                    QKT = pa.tile([128, 12, 512], f8, tag="QKT")
                    for fb in range(12):
                        qk_ps = pxA.tile([128, 512], f32, tag="px")
                        for cc2 in range(CC // 2):
                            nc.tensor.matmul(
                                qk_ps[:],
                                wqk[:, fb, 2 * cc2 : 2 * cc2 + 2, :],
                                hT[:, 2 * cc2 : 2 * cc2 + 2, :],
                                start=(cc2 == 0), stop=(cc2 == CC // 2 - 1),
                                perf_mode=DR,
                            )
                        # sqrt(A16) folded into both q and k so the S matmul
                        # lands A16*S directly in PSUM for the exp bit-trick
                        nc.scalar.activation(
                            out=QKT[:, fb, :], in_=qk_ps[:],
                            func=AF.Identity,
                            bias=sb["bqk"][:, fb : fb + 1],
                            scale=(0.125 * RA16 if fb < 6 else RA16),
                        )
                    # V with a ones column per head: AV emits softmax
                    # denominators (col 64) alongside the head outputs.
                    VG = pa1.tile([128, 4, H, HD + 1], bf16, tag="VG")
                    nc.vector.memset(VG[:, :, :, HD : HD + 1], 1.0)
                    for tl in range(4):
                        v_ps = pxB.tile([128, 2, 512], f32, tag="pxb")
                        for cc2 in range(CC // 2):
                            for nh in range(2):
                                nc.tensor.matmul(
                                    v_ps[:, nh, 0:384],
                                    hT[:, 2 * cc2 : 2 * cc2 + 2, tl * 128 : (tl + 1) * 128],
                                    wv[:, 2 * cc2 : 2 * cc2 + 2, nh * 384 : (nh + 1) * 384],
                                    start=(cc2 == 0), stop=(cc2 == CC // 2 - 1),
                                    perf_mode=DR,
                                )
                        for nh in range(2):
                            nc.scalar.activation(
                                out=VG[:, tl, 6 * nh : 6 * nh + 6, 0:HD],
                                in_=v_ps[:, nh, 0:384].rearrange(
                                    "p (h e) -> p h e", h=6
                                ),
                                func=AF.Copy,
                            )

                    # windows
                    for tl in range(4):
                        t = 4 * g + tl
                        masked = t == NW - 1
                        bfeat_t = sb["bfeatm"] if masked else sb["bfeat"]
                        qs = slice(tl * 128, (tl + 1) * 128)

                        # Even/odd heads target different PSUM banks: MMs with
                        # disjoint PE row-groups (base partition 0 vs 64) run
                        # concurrently, and concurrent writes to one PSUM bank
                        # hard-fault the device. Slot j: even i -> i//2 (bank
                        # 0), odd i -> 4 + i//2 (bank 1).
                        e_feat = []
                        for half in range(2):
                            hh = list(range(half * 6, half * 6 + 6))

                            def _v(t):  # [128,8,128] -> [128,2,3,128] skipping slots 3,7
                                return t[:].rearrange(
                                    "p (g j) k -> p g j k", g=2
                                )[:, :, 0:3, :]

                            s_feat = ps.tile([128, 8, 128], f32, tag="s")
                            for i, h in enumerate(hh):
                                bp = (h % 2) * 64
                                j = (i // 2) + 4 * (i % 2)
                                nc.tensor.matmul(
                                    s_feat[:, j, :],
                                    QKT[bp : bp + 64, 6 + h // 2, qs],
                                    QKT[bp : bp + 64, h // 2, qs],
                                    start=(i in (0, 1)), stop=(i in (4, 5)),
                                )
                            # exp(S)*bias via the Schraudolph bit-trick: PSUM
                            # already holds A16*S; adding the precomputed
                            # A16*log(bias)+B16-C16 table and converting to
                            # int16 yields the bf16 BIT pattern of
                            # exp(S)*bias. Masked keys sit at +8000 → denormal
                            # bf16 ≈ 0. One vector op replaces exp+mul and
                            # keeps the EXP act-table off the scalar engine.
                            E_f = pa.tile([128, 2, 3, 128], i16, tag="E_feat")
                            nc.vector.tensor_tensor(
                                out=E_f[:], in0=_v(s_feat),
                                in1=bfeat_t[:, half, :, :].rearrange(
                                    "p (g j) k -> p g j k", g=2
                                ),
                                op=OP.add,
                            )
                            e_feat.append(E_f)

                        # AV with ones column: O_ps[:, g, hh*65+64] = denom
                        O_ps = ps.tile([128, 2, 512], f32, tag="s")
                        for h in range(H):
                            i = h % 6
                            nc.tensor.matmul(
                                O_ps[:, h // 6, (h % 6) * 65 : (h % 6) * 65 + 65],
                                e_feat[h // 6][:, i % 2, i // 2, :].bitcast(bf16),
                                VG[:, tl, h, :],
                                start=(h in (0, 6)), stop=(h in (5, 11)),
                            )
                        rden = pa.tile([128, 2, 6, 1], f32, tag="rden")
                        nc.vector.reciprocal(
                            out=rden[:],
                            in_=O_ps[:, :, 0:390].rearrange(
                                "p g (h e) -> p g h e", e=65
                            )[:, :, :, 64:65],
                        )
                        Osb = pa.tile([128, DIM], bf16, tag="Osb")
                        for gg in range(2):
                            nc.vector.tensor_tensor(
                                out=Osb[:, gg * 384 : (gg + 1) * 384].rearrange(
                                    "p (h e) -> p h e", h=6
                                ),
                                in0=O_ps[:, gg, 0:390].rearrange(
                                    "p (h e) -> p h e", e=65
                                )[:, :, 0:64],
                                in1=rden[:, gg, :, :].to_broadcast([128, 6, 64]),
                                op=OP.mult,
                            )
                        OTsb = pa.tile([128, CC, 128], f8, tag="OTsb")
                        OT_bf = pa3.tile([128, CC, 128], bf16, tag="OT_bf")
                        nc.scalar.dma_start_transpose(out=OT_bf[:], in_=Osb[:])
                        nc.scalar.activation(
                            out=OTsb[:], in_=OT_bf[:], func=AF.Copy,
                        )
                        pr_ps = pxB.tile([128, 2, 512], f32, tag="pxb")
                        for cc2 in range(CC // 2):
                            for nh in range(2):
                                nc.tensor.matmul(
                                    pr_ps[:, nh, 0:384],
                                    OTsb[:, 2 * cc2 : 2 * cc2 + 2, :],
                                    wproj[:, 2 * cc2 : 2 * cc2 + 2, nh * 384 : (nh + 1) * 384],
                                    start=(cc2 == 0), stop=(cc2 == CC // 2 - 1),
                                    perf_mode=DR,
                                )
                        # branch output straight into the resident rolled
                        # buffer; pass B reads it back in the same order
                        nc.vector.tensor_tensor(
                            out=asb[:, b * NW + t, :].rearrange(
                                "p (a n) -> p a n", a=2
                            ),
                            in0=pr_ps[:, :, 0:384],
                            in1=sb["bproj"][:].rearrange("p (a n) -> p a n", a=2),
                            op=OP.add,
                        )

            def emit_B(b, g):
                # pass B runs entirely in ROLLED token space: LN2/MLP are
                # per-token, so only the final output DMA needs un-rolling
                # (and rolled tiles map to contiguous token ranges anyway).
                x0 = b * N
                if True:
                    hT = pb.tile([128, CC, 512], f8, tag="hT2")
                    mvg = pb.tile([128, NG, 2], f32, tag="mvg2")
                    rstdg = pb.tile([128, NG], f32, tag="rstdg2")
                    x2G = pb.tile([128, NG, DIM], f32, tag="x2G")
                    for tl in range(4):
                        t = 4 * g + tl
                        x_m = pb3.tile([128, DIM], f32, tag="x_m")
                        if t < NW - 1:
                            nc.sync.dma_start(
                                out=x_m[:],
                                in_=p["xs"][x0 + 64 + 128 * t : x0 + 64 + 128 * (t + 1)],
                            )
                        else:
                            nc.vector.memset(x_m[:], 0.0)
                            nc.sync.dma_start(
                                out=x_m[0:16], in_=p["xs"][x0 + 1984 : x0 + 2000]
                            )
                            nc.sync.dma_start(
                                out=x_m[64:128], in_=p["xs"][x0 : x0 + 64]
                            )
                        nc.gpsimd.tensor_add(
                            out=x2G[:, tl, :], in0=x_m[:], in1=asb[:, b * NW + t, :]
                        )
                        stats = pb3.tile([128, 2, 6], f32, tag="ln_stats2")
                        nc.vector.bn_stats(out=stats[:, 0, :], in_=x2G[:, tl, 0:512])
                        nc.vector.bn_stats(out=stats[:, 1, :], in_=x2G[:, tl, 512:768])
                        nc.vector.bn_aggr(out=mvg[:, tl, :], in_=stats[:])
                    newton_rsqrt(pb3, mvg[:, :, 1], rstdg, "nrb")
                    for tl in range(4):
                        z2 = pb3.tile([128, DIM], bf16, tag="z2")
                        nc.vector.tensor_scalar(
                            out=z2[:], in0=x2G[:, tl, :],
                            scalar1=mvg[:, tl, 0:1], scalar2=rstdg[:, tl : tl + 1],
                            op0=OP.subtract, op1=OP.mult,
                        )
                        transpose6(pb3, z2, hT, tl)

                    gT = pg.tile([128, JB, 512], f8, tag="gT")
                    for jb in range(JB):
                        f_ps = pxA.tile([128, 512], f32, tag="px")
                        for cc2 in range(CC // 2):
                            nc.tensor.matmul(
                                f_ps[:],
                                wfc1[:, 2 * cc2 : 2 * cc2 + 2, jb * 128 : (jb + 1) * 128],
                                hT[:, 2 * cc2 : 2 * cc2 + 2, :],
                                start=(cc2 == 0), stop=(cc2 == CC // 2 - 1),
                                perf_mode=DR,
                            )
                        nc.scalar.activation(
                            out=gT[:, jb, :], in_=f_ps[:], func=AF.Gelu,
                            bias=sb["bfc1"][:, jb : jb + 1], scale=1.0,
                        )
                    for tl in range(4):
                        t = 4 * g + tl
                        m_ps = pxB.tile([128, 2, 512], f32, tag="pxb")
                        for hc2 in range(JB // 2):
                            for nh in range(2):
                                nc.tensor.matmul(
                                    m_ps[:, nh, 0:384],
                                    gT[:, 2 * hc2 : 2 * hc2 + 2, tl * 128 : (tl + 1) * 128],
                                    wfc2[:, 2 * hc2 : 2 * hc2 + 2, nh * 384 : (nh + 1) * 384],
                                    start=(hc2 == 0), stop=(hc2 == JB // 2 - 1),
                                    perf_mode=DR,
                                )
                        o_sb = pb3.tile([128, DIM], f32, tag="o_sb")
                        nc.vector.tensor_tensor(
                            out=o_sb[:].rearrange("p (a n) -> p a n", a=2),
                            in0=m_ps[:, :, 0:384],
                            in1=x2G[:, tl, :].rearrange("p (a n) -> p a n", a=2),
                            op=OP.add,
                        )
                        nc.gpsimd.tensor_add(
                            out=o_sb[:], in0=o_sb[:], in1=sb["bfc2x"][:]
                        )
                        if t < NW - 1:
                            nc.sync.dma_start(
                                out=out_t[x0 + 64 + 128 * t : x0 + 64 + 128 * (t + 1)],
                                in_=o_sb[:],
                            )
                        else:
                            nc.sync.dma_start(
                                out=out_t[x0 + 1984 : x0 + 2000], in_=o_sb[0:16]
                            )
                            nc.sync.dma_start(
                                out=out_t[x0 : x0 + 64], in_=o_sb[64:128]
                            )

            # rolled-space B(b,g) depends only on A(b,g)'s asb tiles, so A/B
            # interleave 1:1 — the PE always has the other pass's GEMMs
            # available while a group head's LN chain runs on the vector engine
            emit_A(0, 0)
            emit_A(0, 1)
            for k in range(6):
                emit_B(k // 4, k % 4)
                emit_A((k + 2) // 4, (k + 2) % 4)
            emit_B(1, 2)
            emit_B(1, 3)

    if fix_waits:
        nsplit = _fix_multi_waits(nc, mybir)
        print(f"_fix_multi_waits: split {nsplit} waits", flush=True)
    return nc


# ---------------------------------------------------------------------------
# host preprocessing
# ---------------------------------------------------------------------------

def _bf(x):
    return np.ascontiguousarray(np.asarray(x, np.float32).astype(BF))


F8 = ml_dtypes.float8_e4m3


def _f8(x):
    return np.ascontiguousarray(np.asarray(x, np.float32).astype(F8))


def _precompute(inp):
    qkv_w = np.asarray(inp["qkv_w"], np.float32)
    qkv_b = np.asarray(inp["qkv_b"], np.float32)
    n1w, n1b = np.asarray(inp["norm1_w"], np.float32), np.asarray(inp["norm1_b"], np.float32)
    n2w, n2b = np.asarray(inp["norm2_w"], np.float32), np.asarray(inp["norm2_b"], np.float32)
    proj_w, proj_b = np.asarray(inp["proj_w"], np.float32), np.asarray(inp["proj_b"], np.float32)
    ls1, ls2 = np.asarray(inp["ls1"], np.float32), np.asarray(inp["ls2"], np.float32)
    fc1_w, fc1_b = np.asarray(inp["fc1_w"], np.float32), np.asarray(inp["fc1_b"], np.float32)
    fc2_w, fc2_b = np.asarray(inp["fc2_w"], np.float32), np.asarray(inp["fc2_b"], np.float32)
    rel_bias = np.asarray(inp["rel_bias"], np.float32)

    c = {}
    wqk = _f8(n1w[:, None] * qkv_w[:, : 2 * DIM])           # [768, 1536]
    # fb-major layout [128, 12, CC, 128] so startup DMA slices are contiguous
    c["wqk"] = np.ascontiguousarray(
        wqk.reshape(CC, 128, 12, 128).transpose(1, 2, 0, 3)
    )
    wv = _f8(n1w[:, None] * qkv_w[:, 2 * DIM :])
    c["wv"] = np.ascontiguousarray(wv.reshape(CC, 128, DIM).transpose(1, 0, 2))
    qkvb_f = n1b @ qkv_w + qkv_b
    bqk = qkvb_f[: 2 * DIM].reshape(12, 128).T.astype(np.float32).copy()
    bqk[:, :6] *= 0.125 * RA16
    bqk[:, 6:] *= RA16
    c["bqk"] = np.ascontiguousarray(bqk)
    bv = qkvb_f[2 * DIM :]
    wproj = _f8(proj_w * ls1[None, :])
    c["wproj"] = np.ascontiguousarray(wproj.reshape(CC, 128, DIM).transpose(1, 0, 2))
    c["bproj"] = np.ascontiguousarray(
        np.broadcast_to(((bv @ proj_w + proj_b) * ls1).astype(np.float32), (128, DIM))
    )
    wfc1 = _f8(n2w[:, None] * fc1_w)
    c["wfc1"] = np.ascontiguousarray(wfc1.reshape(CC, 128, MLP).transpose(1, 0, 2))
    c["bfc1"] = np.ascontiguousarray(
        (n2b @ fc1_w + fc1_b).reshape(JB, 128).T.astype(np.float32)
    )
    wfc2 = _f8(fc2_w * ls2[None, :])
    c["wfc2"] = np.ascontiguousarray(wfc2.reshape(JB, 128, DIM).transpose(1, 0, 2))
    c["bfc2x"] = np.ascontiguousarray(
        np.broadcast_to((fc2_b * ls2).astype(np.float32), (128, DIM))
    )

    coords = np.arange(WS)
    rel_idx = coords[None, :] - coords[:, None] + (NPATCH - 1)
    Bmat = rel_bias[rel_idx].transpose(2, 0, 1).astype(np.float32)  # [H, q, k]
    # head order per half: evens then odds (matches S-slot blocks)
    horder = [0, 2, 4, 1, 3, 5]

    def _blocked(mat, mask):  # mat [H, k, q] -> [k, 2, 6, q] f32 exp-trick bias
        t = A16 * mat + K16  # int16 bits of bf16(exp(bias)) once A16*S added
        if mask:
            # masked keys: constant +8000 → bf16 bits in the denormal range
            t[:, 16:64, :] = 8000.0
        out = np.stack(
            [np.stack([t[6 * half + i] for i in horder], 0) for half in range(2)], 0
        )  # [2, 6, k, q]
        return _bf(out.transpose(2, 0, 1, 3))

    c["bfeat"] = _blocked(Bmat.transpose(0, 2, 1), False)
    c["bfeatm"] = _blocked(Bmat.transpose(0, 2, 1), True)
    c["ident"] = _bf(np.eye(128, dtype=np.float32))
    c["ident8"] = _f8(np.eye(128, dtype=np.float32))
    return c


def kernel(**inputs):
    from concourse.bass_utils import run_bass_kernel_spmd

    if "nc" not in _CACHE:
        _CACHE["nc"] = _build()
    nc = _CACHE["nc"]

    c = _precompute(inputs)
    x = np.asarray(inputs["x"], np.float32)  # [16, 2000, 768]
    in_maps = []
    for core in range(NCORES):
        m = dict(c)
        m["xs"] = np.ascontiguousarray(
            x[core * BL : (core + 1) * BL].reshape(TOK, DIM)
        )
        m["xbf"] = m["xs"].astype(BF)
        in_maps.append(m)
    res = run_bass_kernel_spmd(nc, in_maps, core_ids=list(range(NCORES)))
    out = np.stack(
        [res.results[i]["out"].reshape(BL, N, DIM) for i in range(NCORES)]
    ).reshape(B, N, DIM)
    return out.astype(np.float32)



# revision 73
# speedup vs baseline: 1.1962x; 1.1962x over previous
"""Trainium2 Bass kernel for the shifted-window attention block
(nn_Block_6373731467375), SPMD over 8 NeuronCores, data-parallel over batch.

Per core: 2 batch elements. Pass A computes the attention branch in rolled
window space (LN1 folded into qkv weights, dual-S softmax: S token-major for
denominators, S feature-major for the AV matmul), writes the scaled branch
output to a DRAM scratch in original token order. Pass B adds the residual,
applies LN2 (folded into fc1), runs the MLP and writes the final output.
"""

import numpy as np
import ml_dtypes

BF = ml_dtypes.bfloat16

DIM, H, HD, WS, SHIFT, NPATCH, MLP, EPS = 768, 12, 64, 128, 64, 128, 3072, 1e-5
B, N = 16, 2000
NCORES = 8
BL = B // NCORES          # batch elems per core
TOK = BL * N              # 4000
NW = 16                   # rolled 128-token tiles (=windows) per batch elem
NG = 4                    # groups of 4 tiles (512 tokens)
CC = DIM // 128           # 6 contraction chunks
JB = MLP // 128           # 24 hidden blocks
MAGIC = 0x5F3759DF
# Schraudolph exp-in-bf16-bits: bits = A16*x + (127*128 - C16); sqrt(A16) is
# folded into the q/k projections so PSUM holds A16*S directly.
A16 = 128.0 / np.log(2.0)
RA16 = float(np.sqrt(A16))
K16 = 127.0 * 128.0 - 5.5

_CACHE = {}


# ---------------------------------------------------------------------------
# device kernel builder
# ---------------------------------------------------------------------------

def _fix_multi_waits(nc, mybir):
    """This walrus build rejects >1 sync-wait per instruction; hoist extra
    waits onto dedicated NOPs inserted just before, on the same engine."""
    n = 0
    for blk in nc.main_func.blocks:
        new_insts = []
        changed = False
        for ins in blk.instructions:
            si = ins.sync_info
            if si is not None and si.on_wait and len(si.on_wait) > 1:
                waits = list(si.on_wait)
                for w in waits[:-1]:
                    n += 1
                    nop = mybir.InstNoOp(
                        name=f"{ins.name}-sw{n}",
                        engine=ins.engine,
                        ins=[],
                        outs=[],
                        bass_nofuse=True,
                        sync_info=mybir.SyncInfo(on_wait=[w], on_update=[]),
                    )
                    new_insts.append(nop)
                si.on_wait = waits[-1:]
                changed = True
            new_insts.append(ins)
        if changed:
            blk.instructions = new_insts
    return n


def _build(fix_waits=True, passes=('A','B')):
    import concourse.bass as bass
    import concourse.mybir as mybir
    from contextlib import ExitStack

    f32 = mybir.dt.float32
    bf16 = mybir.dt.bfloat16
    f8 = mybir.dt.float8e4
    u32 = mybir.dt.uint32
    i16 = mybir.dt.int16
    DR = mybir.MatmulPerfMode.DoubleRow
    AX = mybir.AxisListType
    OP = mybir.AluOpType
    AF = mybir.ActivationFunctionType

    from concourse.tile import TileContext

    nc = bass.Bass()
    p = {}
    p["xs"] = nc.declare_dram_parameter("xs", [TOK, DIM], f32, isOutput=False)
    p["xbf"] = nc.declare_dram_parameter("xbf", [TOK, DIM], bf16, isOutput=False)
    p["wqk"] = nc.declare_dram_parameter("wqk", [128, 12, CC, 128], f8, isOutput=False)
    p["wv"] = nc.declare_dram_parameter("wv", [128, CC, DIM], f8, isOutput=False)
    p["wproj"] = nc.declare_dram_parameter("wproj", [128, CC, DIM], f8, isOutput=False)
    p["wfc1"] = nc.declare_dram_parameter("wfc1", [128, CC, MLP], f8, isOutput=False)
    p["wfc2"] = nc.declare_dram_parameter("wfc2", [128, JB, DIM], f8, isOutput=False)
    p["bqk"] = nc.declare_dram_parameter("bqk", [128, 12], f32, isOutput=False)
    p["bfc1"] = nc.declare_dram_parameter("bfc1", [128, JB], f32, isOutput=False)
    p["bproj"] = nc.declare_dram_parameter("bproj", [128, DIM], f32, isOutput=False)
    p["bfc2x"] = nc.declare_dram_parameter("bfc2x", [128, DIM], f32, isOutput=False)
    p["bfeat"] = nc.declare_dram_parameter("bfeat", [128, 2, 6, WS], bf16, isOutput=False)
    p["bfeatm"] = nc.declare_dram_parameter("bfeatm", [128, 2, 6, WS], bf16, isOutput=False)
    p["ident"] = nc.declare_dram_parameter("ident", [128, 128], bf16, isOutput=False)
    p["ident8"] = nc.declare_dram_parameter("ident8", [128, 128], f8, isOutput=False)
    out_t = nc.declare_dram_parameter("out", [TOK, DIM], f32, isOutput=True)

    with TileContext(nc) as tc, ExitStack() as ctx:
        cpool = ctx.enter_context(tc.tile_pool(name="consts", bufs=1))

        # resident constants; only the tiny first-needed ones are fetched up
        # front — the window/MLP tables are DMA'd from inside emit_A(0,0)
        # after the first group's x tiles, so the startup burst stays small.
        sb = {}
        for name in ("bqk",):
            t = cpool.tile(list(p[name].shape), p[name].dtype, tag=name)
            nc.sync.dma_start(out=t[:], in_=p[name][:])
            sb[name] = t
        for name in ("bfeat", "bfeatm", "bproj", "bfc1", "bfc2x"):
            sb[name] = cpool.tile(
                list(p[name].shape), p[name].dtype, tag=name, name=name
            )
        magic = cpool.tile([128, 1], u32, tag="magic")
        nc.vector.memset(magic[:], MAGIC)

        wB = ctx.enter_context(tc.tile_pool(name="wB", bufs=1))
        wfc1 = wB.tile([128, CC, MLP], f8)
        wfc2 = wB.tile([128, JB, DIM], f8)
        # attention branch output, resident in rolled token order (f8: the
        # branch is ls1-scaled by 1e-5, so 6% quantization is invisible)
        asb = wB.tile([128, 2 * NW, DIM], f8)

        # ---------------- helpers ----------------
        def newton_rsqrt(pool, var_view, rstdg, tagp):
            """rstdg[:, :NG] = rsqrt(var_view + eps) via 3 fp32 Newton steps."""
            vts = pool.tile([128, NG], f32, tag=tagp + "v")
            y = pool.tile([128, NG], f32, tag=tagp + "y")
            t1 = pool.tile([128, NG], f32, tag=tagp + "t")
            nc.vector.tensor_scalar_add(out=vts[:], in0=var_view, scalar1=EPS)
            nc.vector.tensor_scalar(
                out=y[:].bitcast(u32),
                in0=vts[:].bitcast(u32),
                scalar1=1,
                scalar2=None,
                op0=OP.logical_shift_right,
            )
            nc.vector.tensor_tensor(
                out=y[:].bitcast(u32),
                in0=magic[:].to_broadcast([128, NG]),
                in1=y[:].bitcast(u32),
                op=OP.subtract,
            )
            a, b = y, rstdg
            for _ in range(1):
                nc.vector.tensor_mul(out=t1[:], in0=a[:], in1=a[:])
                nc.vector.tensor_mul(out=t1[:], in0=t1[:], in1=vts[:])
                nc.vector.tensor_scalar(
                    out=t1[:], in0=t1[:], scalar1=-0.5, scalar2=1.5,
                    op0=OP.mult, op1=OP.add,
                )
                nc.vector.tensor_mul(out=b[:], in0=a[:], in1=t1[:])
                a, b = b, a
            assert a is rstdg  # odd iteration count lands in caller's tile

        def transpose12(pool, z_t2, dst, tl0):
            """z_t2 [128,2,768] bf16 (a token-tile pair) -> dst[:, :,
            tl0*128:(tl0+2)*128] via ONE X-bar DMA transpose of the pair —
            halves the transpose instruction count on the sync queue."""
            zT = pool.tile([128, 2, CC, 128], bf16, tag="zTd")
            nc.sync.dma_start_transpose(
                out=zT[:].rearrange("p a c q -> p (a c) q"),
                in_=z_t2[:].rearrange("p a d -> p (a d)"),
            )
            nc.scalar.activation(
                out=dst[:, :, tl0 * 128 : (tl0 + 2) * 128].rearrange(
                    "p c (a q) -> p a c q", a=2
                ),
                in_=zT[:],
                func=AF.Copy,
            )

        # =================== PASSES (A/B interleaved via deps) ==========
        if True:
         with tc.tile_pool(name="wA", bufs=1) as wA, \
             tc.tile_pool(name="pa", bufs=2) as pa, \
             tc.tile_pool(name="pa1", bufs=1) as pa1, \
             tc.tile_pool(name="pa3", bufs=2) as pa3, \
             tc.tile_pool(name="pb", bufs=2) as pb, \
             tc.tile_pool(name="pb3", bufs=2) as pb3, \
             tc.tile_pool(name="pg", bufs=1) as pg, \
             tc.tile_pool(name="pxA", bufs=2, space="PSUM") as pxA, \
             tc.tile_pool(name="pxB", bufs=1, space="PSUM") as pxB, \
             tc.tile_pool(name="ps", bufs=2, space="PSUM") as ps:

            wqk = wA.tile([128, 12, CC, 128], f8)
            wv = wA.tile([128, CC, DIM], f8)
            wproj = wA.tile([128, CC, DIM], f8)

            def emit_A(b, g):
                x0 = b * N
                if True:
                    if b == 0 and g == 1:
                        # prefetch MLP weights; the dummy copies into wfc1/2
                        # give the prefetch DMAs a write-after-write dep on
                        # wv's arrival so they can't steal DMA bandwidth from
                        # the startup-critical x/qkv tiles
                        nc.gpsimd.tensor_copy(
                            out=wfc1[0:1, 0:1, 0:1], in_=wv[0:1, 0:1, 0:1]
                        )
                        nc.gpsimd.tensor_copy(
                            out=wfc2[0:1, 0:1, 0:1], in_=wv[0:1, 0:1, 0:1]
                        )
                        nc.gpsimd.dma_start(out=wfc1[:], in_=p["wfc1"][:])
                        nc.gpsimd.dma_start(out=wfc2[:], in_=p["wfc2"][:])
                    hT = pa.tile([128, CC, 512], f8, tag="hT")
                    first = b == 0 and g == 0
                    mvg = pa.tile([128, NG, 2], f32, tag="mvg")
                    rstdg = pa.tile([128, NG], f32, tag="rstdg")
                    xG = pa.tile([128, NG, DIM], bf16, tag="xG")
                    for tl in range(4):
                        t = 4 * g + tl
                        if t < NW - 1:
                            nc.sync.dma_start(
                                out=xG[:, tl, :],
                                in_=p["xbf"][x0 + 64 + 128 * t : x0 + 64 + 128 * (t + 1)],
                            )
                        else:
                            nc.vector.memset(xG[:, tl, :], 0.0)
                            nc.sync.dma_start(
                                out=xG[0:16, tl, :], in_=p["xbf"][x0 + 1984 : x0 + 2000]
                            )
                            nc.sync.dma_start(
                                out=xG[64:128, tl, :], in_=p["xbf"][x0 : x0 + 64]
                            )
                        stats = pa3.tile([128, 2, 6], f32, tag="ln_stats")
                        nc.vector.bn_stats(out=stats[:, 0, :], in_=xG[:, tl, 0:512])
                        nc.vector.bn_stats(out=stats[:, 1, :], in_=xG[:, tl, 512:768])
                        nc.vector.bn_aggr(out=mvg[:, tl, :], in_=stats[:])
                    if first:
                        # weights follow the first group's x tiles on the queues,
                        # in order of first use: qkv slices, V, window tables,
                        # proj, then the remaining bias tables.
                        for fblk in range(4):
                            nc.sync.dma_start(
                                out=wqk[:, 3 * fblk : 3 * fblk + 3, :, :],
                                in_=p["wqk"][:, 3 * fblk : 3 * fblk + 3, :, :],
                            )
                        nc.sync.dma_start(out=wv[:], in_=p["wv"][:])
                        for name in ("bfeat", "bfeatm"):
                            nc.sync.dma_start(out=sb[name][:], in_=p[name][:])
                        nc.sync.dma_start(out=wproj[:], in_=p["wproj"][:])
                        for name in ("bproj", "bfc1", "bfc2x"):
                            nc.sync.dma_start(out=sb[name][:], in_=p[name][:])
                    newton_rsqrt(pa3, mvg[:, :, 1], rstdg, "nra")
                    for h2 in range(2):
                        z_t = pa3.tile([128, 2, DIM], bf16, tag="z_t")
                        for su in range(2):
                            tl = 2 * h2 + su
                            nc.vector.tensor_scalar(
                                out=z_t[:, su, :], in0=xG[:, tl, :],
                                scalar1=mvg[:, tl, 0:1],
                                scalar2=rstdg[:, tl : tl + 1],
                                op0=OP.subtract, op1=OP.mult,
                            )
                        transpose12(pa3, z_t, hT, 2 * h2)

                    # qkv for the group (f8: feeds only the S matmuls, whose
                    # output goes through the ~3%-accurate exp trick anyway)
                    QKT = pa.tile([128, 12, 512], f8, tag="QKT")
                    for fb in range(12):
                        qk_ps = pxA.tile([128, 512], f32, tag="px")
                        for cc2 in range(CC // 2):
                            nc.tensor.matmul(
                                qk_ps[:],
                                wqk[:, fb, 2 * cc2 : 2 * cc2 + 2, :],
                                hT[:, 2 * cc2 : 2 * cc2 + 2, :],
                                start=(cc2 == 0), stop=(cc2 == CC // 2 - 1),
                                perf_mode=DR,
                            )
                        # sqrt(A16) folded into both q and k so the S matmul
                        # lands A16*S directly in PSUM for the exp bit-trick
                        nc.scalar.activation(
                            out=QKT[:, fb, :], in_=qk_ps[:],
                            func=AF.Identity,
                            bias=sb["bqk"][:, fb : fb + 1],
                            scale=(0.125 * RA16 if fb < 6 else RA16),
                        )
                    # V with a ones column per head: AV emits softmax
                    # denominators (col 64) alongside the head outputs.
                    VG = pa1.tile([128, 4, H, HD + 1], bf16, tag="VG")
                    nc.vector.memset(VG[:, :, :, HD : HD + 1], 1.0)
                    for tl in range(4):
                        v_ps = pxB.tile([128, 2, 512], f32, tag="pxb")
                        for cc2 in range(CC // 2):
                            for nh in range(2):
                                nc.tensor.matmul(
                                    v_ps[:, nh, 0:384],
                                    hT[:, 2 * cc2 : 2 * cc2 + 2, tl * 128 : (tl + 1) * 128],
                                    wv[:, 2 * cc2 : 2 * cc2 + 2, nh * 384 : (nh + 1) * 384],
                                    start=(cc2 == 0), stop=(cc2 == CC // 2 - 1),
                                    perf_mode=DR,
                                )
                        for nh in range(2):
                            nc.scalar.activation(
                                out=VG[:, tl, 6 * nh : 6 * nh + 6, 0:HD],
                                in_=v_ps[:, nh, 0:384].rearrange(
                                    "p (h e) -> p h e", h=6
                                ),
                                func=AF.Copy,
                            )

                    # windows
                    for tl in range(4):
                        t = 4 * g + tl
                        masked = t == NW - 1
                        bfeat_t = sb["bfeatm"] if masked else sb["bfeat"]
                        qs = slice(tl * 128, (tl + 1) * 128)

                        # Even/odd heads target different PSUM banks: MMs with
                        # disjoint PE row-groups (base partition 0 vs 64) run
                        # concurrently, and concurrent writes to one PSUM bank
                        # hard-fault the device. Slot j: even i -> i//2 (bank
                        # 0), odd i -> 4 + i//2 (bank 1).
                        e_feat = []
                        for half in range(2):
                            hh = list(range(half * 6, half * 6 + 6))

                            def _v(t):  # [128,8,128] -> [128,2,3,128] skipping slots 3,7
                                return t[:].rearrange(
                                    "p (g j) k -> p g j k", g=2
                                )[:, :, 0:3, :]

                            s_feat = ps.tile([128, 8, 128], f32, tag="s")
                            for i, h in enumerate(hh):
                                bp = (h % 2) * 64
                                j = (i // 2) + 4 * (i % 2)
                                nc.tensor.matmul(
                                    s_feat[:, j, :],
                                    QKT[bp : bp + 64, 6 + h // 2, qs],
                                    QKT[bp : bp + 64, h // 2, qs],
                                    start=(i in (0, 1)), stop=(i in (4, 5)),
                                )
                            # exp(S)*bias via the Schraudolph bit-trick: PSUM
                            # already holds A16*S; adding the precomputed
                            # A16*log(bias)+B16-C16 table and converting to
                            # int16 yields the bf16 BIT pattern of
                            # exp(S)*bias. Masked keys sit at +8000 → denormal
                            # bf16 ≈ 0. One vector op replaces exp+mul and
                            # keeps the EXP act-table off the scalar engine.
                            E_f = pa.tile([128, 2, 3, 128], i16, tag="E_feat")
                            nc.vector.tensor_tensor(
                                out=E_f[:], in0=_v(s_feat),
                                in1=bfeat_t[:, half, :, :].rearrange(
                                    "p (g j) k -> p g j k", g=2
                                ),
                                op=OP.add,
                            )
                            e_feat.append(E_f)

                        # AV with ones column: O_ps[:, g, hh*65+64] = denom
                        O_ps = ps.tile([128, 2, 512], f32, tag="s")
                        for h in range(H):
                            i = h % 6
                            nc.tensor.matmul(
                                O_ps[:, h // 6, (h % 6) * 65 : (h % 6) * 65 + 65],
                                e_feat[h // 6][:, i % 2, i // 2, :].bitcast(bf16),
                                VG[:, tl, h, :],
                                start=(h in (0, 6)), stop=(h in (5, 11)),
                            )
                        rden = pa.tile([128, 2, 6, 1], f32, tag="rden")
                        nc.vector.reciprocal(
                            out=rden[:],
                            in_=O_ps[:, :, 0:390].rearrange(
                                "p g (h e) -> p g h e", e=65
                            )[:, :, :, 64:65],
                        )
                        Osb = pa.tile([128, DIM], bf16, tag="Osb")
                        for gg in range(2):
                            nc.vector.tensor_tensor(
                                out=Osb[:, gg * 384 : (gg + 1) * 384].rearrange(
                                    "p (h e) -> p h e", h=6
                                ),
                                in0=O_ps[:, gg, 0:390].rearrange(
                                    "p (h e) -> p h e", e=65
                                )[:, :, 0:64],
                                in1=rden[:, gg, :, :].to_broadcast([128, 6, 64]),
                                op=OP.mult,
                            )
                        OTsb = pa.tile([128, CC, 128], f8, tag="OTsb")
                        OT_bf = pa3.tile([128, CC, 128], bf16, tag="OT_bf")
                        nc.sync.dma_start_transpose(out=OT_bf[:], in_=Osb[:])
                        nc.scalar.activation(
                            out=OTsb[:], in_=OT_bf[:], func=AF.Copy,
                        )
                        pr_ps = pxB.tile([128, 2, 512], f32, tag="pxb")
                        for cc2 in range(CC // 2):
                            for nh in range(2):
                                nc.tensor.matmul(
                                    pr_ps[:, nh, 0:384],
                                    OTsb[:, 2 * cc2 : 2 * cc2 + 2, :],
                                    wproj[:, 2 * cc2 : 2 * cc2 + 2, nh * 384 : (nh + 1) * 384],
                                    start=(cc2 == 0), stop=(cc2 == CC // 2 - 1),
                                    perf_mode=DR,
                                )
                        # branch output straight into the resident rolled
                        # buffer; pass B reads it back in the same order
                        nc.vector.tensor_tensor(
                            out=asb[:, b * NW + t, :].rearrange(
                                "p (a n) -> p a n", a=2
                            ),
                            in0=pr_ps[:, :, 0:384],
                            in1=sb["bproj"][:].rearrange("p (a n) -> p a n", a=2),
                            op=OP.add,
                        )

            def emit_B(b, g):
                # pass B runs entirely in ROLLED token space: LN2/MLP are
                # per-token, so only the final output DMA needs un-rolling
                # (and rolled tiles map to contiguous token ranges anyway).
                x0 = b * N
                if True:
                    hT = pb.tile([128, CC, 512], f8, tag="hT2")
                    mvg = pb.tile([128, NG, 2], f32, tag="mvg2")
                    rstdg = pb.tile([128, NG], f32, tag="rstdg2")
                    x2G = pb.tile([128, NG, DIM], f32, tag="x2G")
                    for tl in range(4):
                        t = 4 * g + tl
                        if t < NW - 1:
                            nc.sync.dma_start(
                                out=x2G[:, tl, :],
                                in_=p["xs"][x0 + 64 + 128 * t : x0 + 64 + 128 * (t + 1)],
                            )
                        else:
                            nc.vector.memset(x2G[:, tl, :], 0.0)
                            nc.sync.dma_start(
                                out=x2G[0:16, tl, :], in_=p["xs"][x0 + 1984 : x0 + 2000]
                            )
                            nc.sync.dma_start(
                                out=x2G[64:128, tl, :], in_=p["xs"][x0 : x0 + 64]
                            )
                        nc.gpsimd.tensor_add(
                            out=x2G[:, tl, :], in0=x2G[:, tl, :],
                            in1=asb[:, b * NW + t, :],
                        )
                        stats = pb3.tile([128, 2, 6], f32, tag="ln_stats2")
                        nc.vector.bn_stats(out=stats[:, 0, :], in_=x2G[:, tl, 0:512])
                        nc.vector.bn_stats(out=stats[:, 1, :], in_=x2G[:, tl, 512:768])
                        nc.vector.bn_aggr(out=mvg[:, tl, :], in_=stats[:])
                    newton_rsqrt(pb3, mvg[:, :, 1], rstdg, "nrb")
                    for h2 in range(2):
                        z2 = pb3.tile([128, 2, DIM], bf16, tag="z2")
                        for su in range(2):
                            tl = 2 * h2 + su
                            nc.vector.tensor_scalar(
                                out=z2[:, su, :], in0=x2G[:, tl, :],
                                scalar1=mvg[:, tl, 0:1],
                                scalar2=rstdg[:, tl : tl + 1],
                                op0=OP.subtract, op1=OP.mult,
                            )
                        transpose12(pa3, z2, hT, 2 * h2)

                    gT = pg.tile([128, JB, 512], f8, tag="gT")
                    for jb in range(JB):
                        f_ps = pxA.tile([128, 512], f32, tag="px")
                        for cc2 in range(CC // 2):
                            nc.tensor.matmul(
                                f_ps[:],
                                wfc1[:, 2 * cc2 : 2 * cc2 + 2, jb * 128 : (jb + 1) * 128],
                                hT[:, 2 * cc2 : 2 * cc2 + 2, :],
                                start=(cc2 == 0), stop=(cc2 == CC // 2 - 1),
                                perf_mode=DR,
                            )
                        nc.scalar.activation(
                            out=gT[:, jb, :], in_=f_ps[:], func=AF.Gelu,
                            bias=sb["bfc1"][:, jb : jb + 1], scale=1.0,
                        )
                    for tl in range(4):
                        t = 4 * g + tl
                        m_ps = pxB.tile([128, 2, 512], f32, tag="pxb")
                        for hc2 in range(JB // 2):
                            for nh in range(2):
                                nc.tensor.matmul(
                                    m_ps[:, nh, 0:384],
                                    gT[:, 2 * hc2 : 2 * hc2 + 2, tl * 128 : (tl + 1) * 128],
                                    wfc2[:, 2 * hc2 : 2 * hc2 + 2, nh * 384 : (nh + 1) * 384],
                                    start=(hc2 == 0), stop=(hc2 == JB // 2 - 1),
                                    perf_mode=DR,
                                )
                        o_sb = pb3.tile([128, DIM], f32, tag="o_sb")
                        nc.vector.tensor_tensor(
                            out=o_sb[:].rearrange("p (a n) -> p a n", a=2),
                            in0=m_ps[:, :, 0:384],
                            in1=x2G[:, tl, :].rearrange("p (a n) -> p a n", a=2),
                            op=OP.add,
                        )
                        nc.gpsimd.tensor_add(
                            out=o_sb[:], in0=o_sb[:], in1=sb["bfc2x"][:]
                        )
                        if t < NW - 1:
                            nc.sync.dma_start(
                                out=out_t[x0 + 64 + 128 * t : x0 + 64 + 128 * (t + 1)],
                                in_=o_sb[:],
                            )
                        else:
                            nc.sync.dma_start(
                                out=out_t[x0 + 1984 : x0 + 2000], in_=o_sb[0:16]
                            )
                            nc.sync.dma_start(
                                out=out_t[x0 : x0 + 64], in_=o_sb[64:128]
                            )

            # rolled-space B(b,g) depends only on A(b,g)'s asb tiles, so A/B
            # interleave 1:1 — the PE always has the other pass's GEMMs
            # available while a group head's LN chain runs on the vector engine
            emit_A(0, 0)
            emit_A(0, 1)
            for k in range(6):
                emit_B(k // 4, k % 4)
                emit_A((k + 2) // 4, (k + 2) % 4)
            emit_B(1, 2)
            emit_B(1, 3)

    if fix_waits:
        nsplit = _fix_multi_waits(nc, mybir)
        print(f"_fix_multi_waits: split {nsplit} waits", flush=True)
    return nc


# ---------------------------------------------------------------------------
# host preprocessing
# ---------------------------------------------------------------------------

def _bf(x):
    return np.ascontiguousarray(np.asarray(x, np.float32).astype(BF))


F8 = ml_dtypes.float8_e4m3


def _f8(x):
    return np.ascontiguousarray(np.asarray(x, np.float32).astype(F8))


def _precompute(inp):
    qkv_w = np.asarray(inp["qkv_w"], np.float32)
    qkv_b = np.asarray(inp["qkv_b"], np.float32)
    n1w, n1b = np.asarray(inp["norm1_w"], np.float32), np.asarray(inp["norm1_b"], np.float32)
    n2w, n2b = np.asarray(inp["norm2_w"], np.float32), np.asarray(inp["norm2_b"], np.float32)
    proj_w, proj_b = np.asarray(inp["proj_w"], np.float32), np.asarray(inp["proj_b"], np.float32)
    ls1, ls2 = np.asarray(inp["ls1"], np.float32), np.asarray(inp["ls2"], np.float32)
    fc1_w, fc1_b = np.asarray(inp["fc1_w"], np.float32), np.asarray(inp["fc1_b"], np.float32)
    fc2_w, fc2_b = np.asarray(inp["fc2_w"], np.float32), np.asarray(inp["fc2_b"], np.float32)
    rel_bias = np.asarray(inp["rel_bias"], np.float32)

    c = {}
    wqk = _f8(n1w[:, None] * qkv_w[:, : 2 * DIM])           # [768, 1536]
    # fb-major layout [128, 12, CC, 128] so startup DMA slices are contiguous
    c["wqk"] = np.ascontiguousarray(
        wqk.reshape(CC, 128, 12, 128).transpose(1, 2, 0, 3)
    )
    wv = _f8(n1w[:, None] * qkv_w[:, 2 * DIM :])
    c["wv"] = np.ascontiguousarray(wv.reshape(CC, 128, DIM).transpose(1, 0, 2))
    qkvb_f = n1b @ qkv_w + qkv_b
    bqk = qkvb_f[: 2 * DIM].reshape(12, 128).T.astype(np.float32).copy()
    bqk[:, :6] *= 0.125 * RA16
    bqk[:, 6:] *= RA16
    c["bqk"] = np.ascontiguousarray(bqk)
    bv = qkvb_f[2 * DIM :]
    wproj = _f8(proj_w * ls1[None, :])
    c["wproj"] = np.ascontiguousarray(wproj.reshape(CC, 128, DIM).transpose(1, 0, 2))
    c["bproj"] = np.ascontiguousarray(
        np.broadcast_to(((bv @ proj_w + proj_b) * ls1).astype(np.float32), (128, DIM))
    )
    wfc1 = _f8(n2w[:, None] * fc1_w)
    c["wfc1"] = np.ascontiguousarray(wfc1.reshape(CC, 128, MLP).transpose(1, 0, 2))
    c["bfc1"] = np.ascontiguousarray(
        (n2b @ fc1_w + fc1_b).reshape(JB, 128).T.astype(np.float32)
    )
    wfc2 = _f8(fc2_w * ls2[None, :])
    c["wfc2"] = np.ascontiguousarray(wfc2.reshape(JB, 128, DIM).transpose(1, 0, 2))
    c["bfc2x"] = np.ascontiguousarray(
        np.broadcast_to((fc2_b * ls2).astype(np.float32), (128, DIM))
    )

    coords = np.arange(WS)
    rel_idx = coords[None, :] - coords[:, None] + (NPATCH - 1)
    Bmat = rel_bias[rel_idx].transpose(2, 0, 1).astype(np.float32)  # [H, q, k]
    # head order per half: evens then odds (matches S-slot blocks)
    horder = [0, 2, 4, 1, 3, 5]

    def _blocked(mat, mask):  # mat [H, k, q] -> [k, 2, 6, q] f32 exp-trick bias
        t = A16 * mat + K16  # int16 bits of bf16(exp(bias)) once A16*S added
        if mask:
            # masked keys: constant +8000 → bf16 bits in the denormal range
            t[:, 16:64, :] = 8000.0
        out = np.stack(
            [np.stack([t[6 * half + i] for i in horder], 0) for half in range(2)], 0
        )  # [2, 6, k, q]
        return _bf(out.transpose(2, 0, 1, 3))

    c["bfeat"] = _blocked(Bmat.transpose(0, 2, 1), False)
    c["bfeatm"] = _blocked(Bmat.transpose(0, 2, 1), True)
    c["ident"] = _bf(np.eye(128, dtype=np.float32))
    c["ident8"] = _f8(np.eye(128, dtype=np.float32))
    return c


def kernel(**inputs):
    from concourse.bass_utils import run_bass_kernel_spmd

    if "nc" not in _CACHE:
        _CACHE["nc"] = _build()
    nc = _CACHE["nc"]

    c = _precompute(inputs)
    x = np.asarray(inputs["x"], np.float32)  # [16, 2000, 768]
    in_maps = []
    for core in range(NCORES):
        m = dict(c)
        m["xs"] = np.ascontiguousarray(
            x[core * BL : (core + 1) * BL].reshape(TOK, DIM)
        )
        m["xbf"] = m["xs"].astype(BF)
        in_maps.append(m)
    res = run_bass_kernel_spmd(nc, in_maps, core_ids=list(range(NCORES)))
    out = np.stack(
        [res.results[i]["out"].reshape(BL, N, DIM) for i in range(NCORES)]
    ).reshape(B, N, DIM)
    return out.astype(np.float32)

